# revision 2
# baseline (speedup 1.0000x reference)
"""Trainium2 Bass kernel for nn_Conversation_Self_Attention.

Reference math (B=64, S=D=DK=DV=512):
    Q = X Wq^T + bq ; K = X Wk^T + bk ; V = X Wv^T + bv
    Uq = P Wpq^T + bpq ; Uk = P Wpk^T + bpk
    att = (Q K^T + Uq Uk^T) * norm + bias      (bias[t] broadcasts over rows)
    att_sm = softmax(att, axis=-1)
    out[i,j] = sum_t att_sm[i,t] V[j,t]

Algebraic reduction used here (saves 2 of 8 cube matmuls):
    Q K^T  = X (Wq^T Wk) X^T + (X Wq^T bk) 1^T + 1 (X Wk^T bq)^T + (bq.bk) 1 1^T
The terms constant along the softmax axis cancel in softmax, so with
W1 = Wq^T Wk and g1 = Wk^T bq:
    softmax(att) == softmax(norm * (X W1 + 1 g1^T) X^T + ... + bias 1^T-ish)
Same for the positional term with W2 = Wpq^T Wpk, g2 = Wpk^T bpq.

Per core (8 batches), per batch, all PE matmuls in bf16:
    X^T, P^T     via PE transposes (bf16, identity)
    T1T = W1^T-chunks x X^T   [d',s]  (+ g1 outer product if nonzero)
    T2T likewise from P^T
    M   = Wv X^T (+ bv outer) = V^T  [t,j]
    att_psum[s-chunk] = sum_k T1T^T X^T + T2T^T P^T + ones^T (bias/norm)
    exp = Exp(norm * att_psum)  on ACT with fused row-sum accum_out
    att_sm = exp * (1/rowsum)   (DVE, per-partition scalar)
    expT = PE-transpose(exp_bf16)
    out = (expT^T x M) * (1/rowsum)  (row rescale on ACT)
"""

import dataclasses
import os

import numpy as np
import ml_dtypes

import concourse.bass as bass
import concourse.tile as tile
import concourse.mybir as mybir
from concourse import masks

B, S, D = 64, 512, 512
NCORES = 8
BPC = B // NCORES  # batches per core
P = 128
NCH = S // P  # 128-row chunks per 512 dim
F32 = mybir.dt.float32
BF16 = mybir.dt.bfloat16
NORM = 1.0 / np.sqrt(2.0 * D).astype(np.float32)
BF = ml_dtypes.bfloat16


# Walrus in this container accepts only a limited number of sem-waits per
# instruction (DMA ≤ 2, CTRL-class like Drain/NoOp fewer). Hoist excess waits
# onto NoOp carrier instructions on the same (in-order) engine sequencer,
# which is semantically equivalent.
_WAIT_CAPS = {}
_DEFAULT_WAIT_CAP = 1


def _split_multiwait(nc):
    for fn in nc.m.functions:
        for bb in fn.blocks:
            insts = bb.instructions
            i = 0
            while i < len(insts):
                inst = insts[i]
                si = getattr(inst, "sync_info", None)
                cap = _WAIT_CAPS.get(type(inst).__name__, _DEFAULT_WAIT_CAP)
                if si is not None and si.on_wait and len(si.on_wait) > cap:
                    waits = list(si.on_wait)
                    pre = [
                        mybir.InstNoOp(
                            name=f"{inst.name}-w{j}",
                            opcode="NoOp",
                            engine=inst.engine,
                            debug=inst.debug,
                            ins=[],
                            outs=[],
                            descendants=None,
                            sync_info=mybir.SyncInfo(on_wait=[w], on_update=[]),
                        )
                        for j, w in enumerate(waits[:-cap])
                    ]
                    inst.sync_info = mybir.SyncInfo(
                        on_wait=waits[-cap:], on_update=list(si.on_update)
                    )
                    insts[i:i] = pre
                    i += len(pre)
                i += 1


def ts(i, n):
    return bass.ts(i, n)


def _build_program(with_gv: bool, repeat: int = 1):
    nc = bass.Bass("TRN2", target_bir_lowering=False, debug=False)

    x_d = nc.dram_tensor("x", [BPC, S, D], BF16, kind="ExternalInput").ap()
    p_d = nc.dram_tensor("p", [BPC, S, D], BF16, kind="ExternalInput").ap()
    w1_d = nc.dram_tensor("w1", [D, D], BF16, kind="ExternalInput").ap()
    w2_d = nc.dram_tensor("w2", [D, D], BF16, kind="ExternalInput").ap()
    wvt_d = nc.dram_tensor("wvt", [D, D], BF16, kind="ExternalInput").ap()
    br_d = nc.dram_tensor("biasrow", [1, S], BF16, kind="ExternalInput").ap()
    ones_d = nc.dram_tensor("onesrow", [1, S], BF16, kind="ExternalInput").ap()
    gv_d = None
    if with_gv:
        gv_d = nc.dram_tensor("gv", [1, 3 * D], BF16, kind="ExternalInput").ap()
    att_d = nc.dram_tensor("att", [BPC, S, S], F32, kind="ExternalOutput").ap()
    out_d = nc.dram_tensor("out", [BPC, S, S], F32, kind="ExternalOutput").ap()

    from contextlib import ExitStack

    with tile.TileContext(nc) as tc, ExitStack() as ctx:
        const = ctx.enter_context(tc.tile_pool(name="const", bufs=1))

        # identity first: gpsimd executes in order and the first PE transpose
        # needs it — it must not queue behind the weight DMAs below
        ident = const.tile([P, P], BF16, tag="ident")
        masks.make_identity(nc, ident[:])

        # weights ride the SWDGE queues so the batch-0 input loads own the
        # HWDGE queues at kernel start (PE's first work is transposing those)
        def load_w(name, dram):
            tiles = []
            for k in range(NCH):
                t = const.tile([P, S], BF16, tag=f"{name}{k}")
                nc.gpsimd.dma_start(t[:], dram[ts(k, P), :])
                tiles.append(t)
            return tiles

        w1_sb = load_w("w1", w1_d)
        w2_sb = load_w("w2", w2_d)
        wvt_sb = load_w("wvt", wvt_d)
        br_sb = const.tile([1, S], BF16, tag="br")
        nc.sync.dma_start(br_sb[:], br_d[:, :])
        ones_sb = const.tile([1, S], BF16, tag="ones")
        nc.sync.dma_start(ones_sb[:], ones_d[:, :])
        gv_sb = None
        if with_gv:
            gv_sb = const.tile([1, 3 * D], BF16, tag="gv")
            nc.sync.dma_start(gv_sb[:], gv_d[:, :])

        # working pools
        xn_pool = ctx.enter_context(tc.tile_pool(name="xn", bufs=12))
        xt_pool = ctx.enter_context(tc.tile_pool(name="xt", bufs=2))
        tmm_pool = ctx.enter_context(tc.tile_pool(name="tmm", bufs=2))
        exp_pool = ctx.enter_context(tc.tile_pool(name="exp", bufs=2))
        zr_pool = ctx.enter_context(tc.tile_pool(name="zr", bufs=8))
        osb_pool = ctx.enter_context(tc.tile_pool(name="osb", bufs=2))

        ps_tp = ctx.enter_context(tc.tile_pool(name="pstp", bufs=2, space="PSUM"))
        ps_mm = ctx.enter_context(tc.tile_pool(name="psmm", bufs=2, space="PSUM"))
        ps_att = ctx.enter_context(tc.tile_pool(name="psatt", bufs=2, space="PSUM"))
        ps_out = ctx.enter_context(tc.tile_pool(name="psout", bufs=2, space="PSUM"))

        # ---- load natural bf16 rows, PE-transpose into XT[i] [d, s]
        def load_rows(dram, b):
            rows = []
            for j in range(NCH):
                xn = xn_pool.tile([P, S], BF16, tag="xn", name="xn")
                nc.sync.dma_start(xn[:], dram[b, ts(j, P), :])
                rows.append(xn)
            return rows

        def transpose_rows(rows, tag):
            tiles = [
                xt_pool.tile([P, S], BF16, tag=f"{tag}{i}", name=f"{tag}{i}")
                for i in range(NCH)
            ]
            for j in range(NCH):  # s-chunks
                for i in range(NCH):  # d-chunks
                    tp = ps_tp.tile([P, P], BF16, tag="tp")
                    nc.tensor.transpose(tp[:], rows[j][:, ts(i, P)], ident[:])
                    nc.vector.tensor_copy(tiles[i][:, ts(j, P)], tp[:])
            return tiles

        batches = [b for _ in range(repeat) for b in range(BPC)]
        # prefetch batch 0's rows ahead of the weight DMAs in program order
        pre_rows = {0: (load_rows(x_d, batches[0]), load_rows(p_d, batches[0]))}

        for bi, b in enumerate(batches):
            x_rows, p_rows = pre_rows.pop(bi) if bi in pre_rows else (
                load_rows(x_d, b), load_rows(p_d, b)
            )
            XT = transpose_rows(x_rows, "xt")
            PT = transpose_rows(p_rows, "pt")

            # ---- T1T / T2T / M
            def proj(w_sb, src, tag, gv_off):
                tiles = []
                for j in range(NCH):
                    ps = ps_mm.tile([P, S], F32, tag="mm")
                    for k in range(NCH):
                        nc.tensor.matmul(
                            ps[:],
                            w_sb[k][:, ts(j, P)],
                            src[k][:],
                            start=(k == 0),
                            stop=(k == NCH - 1 and gv_off is None),
                        )
                    if gv_off is not None:
                        nc.tensor.matmul(
                            ps[:],
                            gv_sb[:, gv_off + j * P : gv_off + (j + 1) * P],
                            ones_sb[:, :],
                            start=False,
                            stop=True,
                        )
                    t = tmm_pool.tile([P, S], BF16, tag=f"{tag}{j}")
                    nc.scalar.copy(t[:], ps[:])
                    tiles.append(t)
                return tiles

            T1T = proj(w1_sb, XT, "t1t", 0 if with_gv else None)
            T2T = proj(w2_sb, PT, "t2t", D if with_gv else None)
            M = proj(wvt_sb, XT, "vt", 2 * D if with_gv else None)

            # ---- attention scores + softmax (rowmax-free: |att| is small)
            expT = [
                xt_pool.tile([P, S], BF16, tag=f"expT{i}", name=f"expT{i}")
                for i in range(NCH)
            ]
            rs = []
            for i in range(NCH):  # s-chunks
                ps = ps_att.tile([P, S], F32, tag="att")
                for k in range(NCH):
                    nc.tensor.matmul(
                        ps[:], T1T[k][:, ts(i, P)], XT[k][:],
                        start=(k == 0), stop=False,
                    )
                for k in range(NCH):
                    nc.tensor.matmul(
                        ps[:], T2T[k][:, ts(i, P)], PT[k][:],
                        start=False, stop=False,
                    )
                nc.tensor.matmul(
                    ps[:], ones_sb[:, :P], br_sb[:, :], start=False, stop=True
                )
                expf = exp_pool.tile([P, S], F32, tag="expf")
                z = zr_pool.tile([P, 1], F32, tag="z")
                nc.scalar.activation(
                    expf[:], ps[:], mybir.ActivationFunctionType.Exp,
                    scale=float(NORM), accum_out=z[:],
                )
                expb = exp_pool.tile([P, S], BF16, tag="expb")
                nc.scalar.activation(
                    expb[:], ps[:], mybir.ActivationFunctionType.Exp,
                    scale=float(NORM),
                )
                r = zr_pool.tile([P, 1], F32, tag="r")
                nc.vector.reciprocal(r[:], z[:])
                rs.append(r)
                asm = exp_pool.tile([P, S], F32, tag="asm")
                nc.vector.tensor_scalar_mul(asm[:], expf[:], r[:])
                nc.sync.dma_start(att_d[b, ts(i, P), :], asm[:])
                for t in range(NCH):  # PE-transpose exp chunk into expT
                    tp = ps_tp.tile([P, P], BF16, tag="tp")
                    nc.tensor.transpose(tp[:], expb[:, ts(t, P)], ident[:])
                    nc.vector.tensor_copy(expT[t][:, ts(i, P)], tp[:])

            # ---- out = diag(r) expT^T M
            for i in range(NCH):
                ps = ps_out.tile([P, S], F32, tag="out")
                for t in range(NCH):
                    nc.tensor.matmul(
                        ps[:], expT[t][:, ts(i, P)], M[t][:],
                        start=(t == 0), stop=(t == NCH - 1),
                    )
                osb = osb_pool.tile([P, S], F32, tag="osb")
                nc.scalar.activation(
                    osb[:], ps[:], mybir.ActivationFunctionType.Copy,
                    scale=rs[i][:],
                )
                nc.sync.dma_start(out_d[b, ts(i, P), :], osb[:])

    _split_multiwait(nc)
    return nc


_prog_cache = {}


def _get_program(with_gv: bool, repeat: int = 1):
    key = (with_gv, repeat)
    if key not in _prog_cache:
        _prog_cache[key] = _build_program(with_gv, repeat)
    return _prog_cache[key]


def _make_runner(nc, donate=True):
    """Persistent jitted SPMD runner (mirrors bass2jax.run_bass_via_pjrt but
    caches the jax.jit so repeat calls don't re-lower/re-compile)."""
    import jax
    from jax.experimental.shard_map import shard_map
    from jax.sharding import Mesh, PartitionSpec
    from concourse.bass2jax import (
        _bass_exec_p,
        install_neuronx_cc_hook,
        partition_id_tensor,
    )

    install_neuronx_cc_hook()
    partition_name = (
        nc.partition_id_tensor.name if nc.partition_id_tensor else None
    )
    in_names, out_names, out_avals, out_shapes = [], [], [], []
    for alloc in nc.m.functions[0].allocations:
        if not isinstance(alloc, mybir.MemoryLocationSet):
            continue
        name = alloc.memorylocations[0].name
        if alloc.kind == "ExternalInput":
            if name != partition_name:
                in_names.append(name)
        elif alloc.kind == "ExternalOutput":
            shape = tuple(alloc.tensor_shape)
            dtype = mybir.dt.np(alloc.dtype)
            out_names.append(name)
            out_avals.append(jax.core.ShapedArray(shape, dtype))
            out_shapes.append((shape, dtype))
    n_params = len(in_names)
    all_in_names = list(in_names) + list(out_names)
    if partition_name is not None:
        all_in_names.append(partition_name)
    donate = tuple(range(n_params, n_params + len(out_names))) if donate else ()

    def _body(*args):
        operands = list(args)
        if partition_name is not None:
            operands.append(partition_id_tensor())
        outs = _bass_exec_p.bind(
            *operands,
            out_avals=tuple(out_avals),
            in_names=tuple(all_in_names),
            out_names=tuple(out_names),
            lowering_input_output_aliases=(),
            sim_require_finite=True,
            sim_require_nnan=True,
            nc=nc,
        )
        return tuple(outs)

    devices = jax.devices()[:NCORES]
    mesh = Mesh(np.asarray(devices), ("core",))
    in_specs = (PartitionSpec("core"),) * (n_params + len(out_names))
    out_specs = (PartitionSpec("core"),) * len(out_names)
    sharded = jax.jit(
        shard_map(
            _body, mesh=mesh, in_specs=in_specs, out_specs=out_specs,
            check_rep=False,
        ),
        donate_argnums=donate if donate else (),
        keep_unused=True,
    )

    def prep(in_maps):
        per_core = [[np.asarray(m[name]) for name in in_names] for m in in_maps]
        concat_in = [
            np.concatenate([per_core[c][i] for c in range(NCORES)], axis=0)
            for i in range(n_params)
        ]
        concat_zeros = [
            np.zeros((NCORES * s[0], *s[1:]), d) for (s, d) in out_shapes
        ]
        return concat_in, concat_zeros

    def run(in_maps, as_numpy=True):
        concat_in, concat_zeros = prep(in_maps)
        out_arrs = sharded(*concat_in, *concat_zeros)
        if not as_numpy:
            jax.block_until_ready(out_arrs)
            return None
        return {n: np.asarray(out_arrs[i]) for i, n in enumerate(out_names)}

    run.sharded = sharded
    run.prep = prep
    run.mesh = mesh
    run.out_names = out_names
    return run


_runner_cache = {}


def _get_runner(with_gv: bool, repeat: int = 1):
    key = (with_gv, repeat)
    if key not in _runner_cache:
        _runner_cache[key] = _make_runner(_get_program(with_gv, repeat))
    return _runner_cache[key]


def _prepare(
    sent_emb, pos_emb, branch_emb,
    Wq, bq, Wk, bk, Wv, bv, Wpq, bpq, Wpk, bpk, bias,
):
    x = np.ascontiguousarray(np.asarray(sent_emb, dtype=np.float32)).astype(BF)
    p = np.ascontiguousarray(np.asarray(pos_emb, dtype=np.float32)).astype(BF)
    Wq = np.asarray(Wq, np.float32); Wk = np.asarray(Wk, np.float32)
    Wv = np.asarray(Wv, np.float32)
    Wpq = np.asarray(Wpq, np.float32); Wpk = np.asarray(Wpk, np.float32)
    bq = np.asarray(bq, np.float32); bk = np.asarray(bk, np.float32)
    bv = np.asarray(bv, np.float32)
    bpq = np.asarray(bpq, np.float32); bpk = np.asarray(bpk, np.float32)
    bias = np.asarray(bias, np.float32)

    W1 = (Wq.T @ Wk).astype(BF)
    W2 = (Wpq.T @ Wpk).astype(BF)
    wvt = np.ascontiguousarray(Wv.T).astype(BF)
    biasrow = (bias / NORM)[None, :].astype(BF)
    onesrow = np.ones((1, S), BF)

    g1 = Wk.T @ bq
    g2 = Wpk.T @ bpq
    with_gv = bool(np.any(g1) or np.any(g2) or np.any(bv))
    gv = np.concatenate([g1, g2, bv])[None, :].astype(BF)

    in_maps = []
    for c in range(NCORES):
        m = {
            "x": x[c * BPC : (c + 1) * BPC],
            "p": p[c * BPC : (c + 1) * BPC],
            "w1": W1, "w2": W2, "wvt": wvt,
            "biasrow": biasrow, "onesrow": onesrow,
        }
        if with_gv:
            m["gv"] = gv
        in_maps.append(m)
    return with_gv, in_maps


def kernel(**inputs):
    with_gv, in_maps = _prepare(**inputs)
    run = _get_runner(with_gv)
    outs = run(in_maps)
    return outs["att"], outs["out"]



# revision 13
# speedup vs baseline: 7.8911x; 7.8911x over previous
"""Trainium2 Bass kernel for nn_Conversation_Self_Attention.

Reference math (B=64, S=D=DK=DV=512):
    Q = X Wq^T + bq ; K = X Wk^T + bk ; V = X Wv^T + bv
    Uq = P Wpq^T + bpq ; Uk = P Wpk^T + bpk
    att = (Q K^T + Uq Uk^T) * norm + bias      (bias[t] broadcasts over rows)
    att_sm = softmax(att, axis=-1)
    out[i,j] = sum_t att_sm[i,t] V[j,t]

Algebraic reduction (saves 2 of 8 cube matmuls):
    softmax(Q K^T + ...) == softmax(norm * X (Wq^T Wk) X^T + ... ) because the
    terms constant along the softmax axis cancel. With W1 = Wq^T Wk and
    g1 = Wk^T bq (nonzero-bias correction), same for the positional term.

Per core (8 batches). Host pre-transposes X, P to XT/PT [d, s] layout so the
device does zero input transposes. Per batch, all PE matmuls bf16:
    T1T[d',s] = sum_k W1[k,d']^T XT[k]          (16 MM)
    T2T, M=V^T likewise from PT / XT            (32 MM)
    att_ps[i] = sum_k T1T[k][:,i]^T XT[k] + T2T[k][:,i]^T PT[k] + 1^T biasrow
    expb[i] = Exp(norm * att_ps[i])  bf16, ACT, accum_out -> z
    r = 1/z (DVE);  asm = expb * r f32 (DVE) -> DMA att out
    tp[i] = PE-transpose(expb[i]) 4x [128,128] blocks into one PSUM tile
    E[i] = DVE copy tp[i] -> SBUF
    out_ps[i] = sum_k E[i][:,k]^T M[k]  ;  osb = Copy(out_ps, scale=r) -> DMA

Software pipeline per chunk i: tp[i-1] | att[i] | out[i-1] so the PE never
waits on the ACT exp chain; the last chunk's tp/out spill into the next
batch's projection phase.
"""

import dataclasses
import os

import numpy as np
import ml_dtypes

import concourse.bass as bass
import concourse.tile as tile
import concourse.mybir as mybir
from concourse import masks

B, S, D = 64, 512, 512
NCORES = 8
BPC = B // NCORES  # batches per core
P = 128
NCH = S // P  # 128-row chunks per 512 dim
F32 = mybir.dt.float32
BF16 = mybir.dt.bfloat16
NORM = 1.0 / np.sqrt(2.0 * D).astype(np.float32)
BF = ml_dtypes.bfloat16
# XBAR DMA-transpose for the exp tiles was measured 305-310us vs 212us for
# PE-mode transposes (the xbar transfer latency stalls the out matmuls and
# HAM-rethrottles the PE), so it stays off by default.
USE_XBAR = os.environ.get("XBAR", "0") == "1"


# Walrus in this container accepts only a limited number of sem-waits per
# instruction (DMA <= 2, CTRL-class like Drain/NoOp fewer). Hoist excess waits
# onto NoOp carrier instructions on the same (in-order) engine sequencer,
# which is semantically equivalent.
_WAIT_CAPS = {}
_DEFAULT_WAIT_CAP = 1


def _split_multiwait(nc):
    for fn in nc.m.functions:
        for bb in fn.blocks:
            insts = bb.instructions
            i = 0
            while i < len(insts):
                inst = insts[i]
                si = getattr(inst, "sync_info", None)
                cap = _WAIT_CAPS.get(type(inst).__name__, _DEFAULT_WAIT_CAP)
                if si is not None and si.on_wait and len(si.on_wait) > cap:
                    waits = list(si.on_wait)
                    pre = [
                        mybir.InstNoOp(
                            name=f"{inst.name}-w{j}",
                            opcode="NoOp",
                            engine=inst.engine,
                            debug=inst.debug,
                            ins=[],
                            outs=[],
                            descendants=None,
                            sync_info=mybir.SyncInfo(on_wait=[w], on_update=[]),
                        )
                        for j, w in enumerate(waits[:-cap])
                    ]
                    inst.sync_info = mybir.SyncInfo(
                        on_wait=waits[-cap:], on_update=list(si.on_update)
                    )
                    insts[i:i] = pre
                    i += len(pre)
                i += 1


def ts(i, n):
    return bass.ts(i, n)


def _build_program(with_gv: bool, repeat: int = 1):
    nc = bass.Bass("TRN2", target_bir_lowering=False, debug=False)

    xt_d = nc.dram_tensor("xt", [BPC, D, S], BF16, kind="ExternalInput").ap()
    pt_d = nc.dram_tensor("pt", [BPC, D, S], BF16, kind="ExternalInput").ap()
    w1_d = nc.dram_tensor("w1", [D, D], BF16, kind="ExternalInput").ap()
    w2_d = nc.dram_tensor("w2", [D, D], BF16, kind="ExternalInput").ap()
    wvt_d = nc.dram_tensor("wvt", [D, D], BF16, kind="ExternalInput").ap()
    br_d = nc.dram_tensor("biasrow", [1, S], BF16, kind="ExternalInput").ap()
    ones_d = nc.dram_tensor("onesrow", [1, S], BF16, kind="ExternalInput").ap()
    gv_d = None
    if with_gv:
        gv_d = nc.dram_tensor("gv", [1, 3 * D], BF16, kind="ExternalInput").ap()
    att_d = nc.dram_tensor("att", [BPC, S, S], F32, kind="ExternalOutput").ap()
    out_d = nc.dram_tensor("out", [BPC, S, S], F32, kind="ExternalOutput").ap()

    from contextlib import ExitStack

    with tile.TileContext(nc) as tc, ExitStack() as ctx:
        const = ctx.enter_context(tc.tile_pool(name="const", bufs=1))

        # identity first: gpsimd executes in order and the first PE transpose
        # needs it - it must not queue behind the weight DMAs below
        ident = None
        if not USE_XBAR:
            ident = const.tile([P, P], BF16, tag="ident")
            masks.make_identity(nc, ident[:])

        # weights ride the SWDGE queues so the batch-0 input loads own the
        # HWDGE queues at kernel start
        def load_w(name, dram):
            tiles = []
            for k in range(NCH):
                t = const.tile([P, S], BF16, tag=f"{name}{k}")
                nc.gpsimd.dma_start(t[:], dram[ts(k, P), :])
                tiles.append(t)
            return tiles

        w1_sb = load_w("w1", w1_d)
        w2_sb = load_w("w2", w2_d)
        wvt_sb = load_w("wvt", wvt_d)
        br_sb = const.tile([1, S], BF16, tag="br")
        nc.gpsimd.dma_start(br_sb[:], br_d[:, :])
        ones_sb = const.tile([1, S], BF16, tag="ones")
        nc.gpsimd.dma_start(ones_sb[:], ones_d[:, :])
        gv_sb = None
        if with_gv:
            gv_sb = const.tile([1, 3 * D], BF16, tag="gv")
            nc.gpsimd.dma_start(gv_sb[:], gv_d[:, :])

        # working pools
        xt_pool = ctx.enter_context(tc.tile_pool(name="xt", bufs=2))
        tmm_pool = ctx.enter_context(tc.tile_pool(name="tmm", bufs=2))
        exp_pool = ctx.enter_context(tc.tile_pool(name="exp", bufs=2))
        e_pool = ctx.enter_context(tc.tile_pool(name="e", bufs=2))
        osb_pool = ctx.enter_context(tc.tile_pool(name="osb", bufs=2))
        zr_pool = ctx.enter_context(tc.tile_pool(name="zr", bufs=2))

        nb = 3 if USE_XBAR else 2  # ps_tp's bank freed in XBAR mode
        ps_mm = ctx.enter_context(tc.tile_pool(name="psmm", bufs=2, space="PSUM"))
        ps_att = ctx.enter_context(tc.tile_pool(name="psatt", bufs=nb, space="PSUM"))
        ps_out = ctx.enter_context(tc.tile_pool(name="psout", bufs=nb, space="PSUM"))
        ps_tp = None
        if not USE_XBAR:
            ps_tp = ctx.enter_context(
                tc.tile_pool(name="pstp", bufs=2, space="PSUM")
            )

        def load_batch(b):
            XT, PT = [], []
            for j in range(NCH):
                t = xt_pool.tile([P, S], BF16, tag=f"xt{j}", name=f"xt{j}")
                nc.sync.dma_start(t[:], xt_d[b, ts(j, P), :])
                XT.append(t)
            for j in range(NCH):
                t = xt_pool.tile([P, S], BF16, tag=f"pt{j}", name=f"pt{j}")
                nc.sync.dma_start(t[:], pt_d[b, ts(j, P), :])
                PT.append(t)
            return XT, PT

        batches = [b for _ in range(repeat) for b in range(BPC)]
        cur = load_batch(batches[0])

        # deferred tail (tp / out of each batch's last chunk)
        tail_tp = [None]
        tail_out = [None]

        for bi, b in enumerate(batches):
            XT, PT = cur
            if bi + 1 < len(batches):
                cur = load_batch(batches[bi + 1])

            # previous batch's last-chunk transposes fill the exp3 latency gap
            if tail_tp[0] is not None:
                tail_tp[0]()
                tail_tp[0] = None

            def proj(w_sb, src, tag, gv_off, copy_fns, after_first=None):
                tiles = []
                for j in range(NCH):
                    ps = ps_mm.tile([P, S], F32, tag="mm")
                    for k in range(NCH):
                        nc.tensor.matmul(
                            ps[:],
                            w_sb[k][:, ts(j, P)],
                            src[k][:],
                            start=(k == 0),
                            stop=(k == NCH - 1 and gv_off is None),
                        )
                    if gv_off is not None:
                        nc.tensor.matmul(
                            ps[:],
                            gv_sb[:, gv_off + j * P : gv_off + (j + 1) * P],
                            ones_sb[:, :],
                            start=False,
                            stop=True,
                        )
                    t = tmm_pool.tile([P, S], BF16, tag=f"{tag}{j}")
                    copy_fns[j](t[:], ps[:])
                    tiles.append(t)
                    if j == (2 if USE_XBAR else 0) and after_first is not None:
                        after_first()
                return tiles

            dve_cp = nc.vector.tensor_copy
            act_cp = nc.scalar.copy
            T1T = proj(
                w1_sb, XT, "t1t", 0 if with_gv else None,
                [dve_cp] * NCH, after_first=tail_out[0],
            )
            tail_out[0] = None
            T2T = proj(w2_sb, PT, "t2t", D if with_gv else None, [act_cp] * NCH)
            M = proj(
                wvt_sb, XT, "vt", 2 * D if with_gv else None,
                [dve_cp, act_cp, dve_cp, act_cp],
            )

            def make_tp(i, expb):
                def emit():
                    if USE_XBAR:
                        # one-shot XBAR block transpose on the ACT hwdge
                        # queue: no PE work, no PSUM, no DVE copy
                        E = e_pool.tile([P, NCH, P], BF16, tag=f"e{i}")
                        nc.scalar.dma_start_transpose(E[:, :, :], expb[:])
                        return [E[:, k, :] for k in range(NCH)]
                    E = e_pool.tile([P, S], BF16, tag=f"e{i}")
                    tp = ps_tp.tile([P, S], BF16, tag="tp")
                    for k in range(NCH):
                        nc.tensor.transpose(
                            tp[:, ts(k, P)], expb[:, ts(k, P)], ident[:]
                        )
                    nc.vector.tensor_copy(E[:], tp[:])
                    return [E[:, ts(k, P)] for k in range(NCH)]

                return emit

            def make_out(i, b, r, M=M):
                def emit(E):
                    ps = ps_out.tile([P, S], F32, tag="out")
                    for k in range(NCH):
                        nc.tensor.matmul(
                            ps[:], E[k], M[k][:],
                            start=(k == 0), stop=(k == NCH - 1),
                        )
                    osb = osb_pool.tile([P, S], F32, tag=f"osb{i % 2}")
                    nc.scalar.activation(
                        osb[:], ps[:], mybir.ActivationFunctionType.Copy,
                        scale=r[:],
                    )
                    nc.sync.dma_start(out_d[b, ts(i, P), :], osb[:])

                return emit

            prev = None  # PE-transpose mode: (tp_emitter, out_emitter)
            pend = []  # XBAR mode: pending out emitters (2-deep pipeline)
            for i in range(NCH):
                # PE mode: previous chunk's transposes + E copy before this
                # chunk's att matmuls (PE order), out matmuls after
                E_prev = None
                if not USE_XBAR and prev is not None:
                    E_prev = prev[0]()

                ps = ps_att.tile([P, S], F32, tag="att")
                for k in range(NCH):
                    nc.tensor.matmul(
                        ps[:], T1T[k][:, ts(i, P)], XT[k][:],
                        start=(k == 0), stop=False,
                    )
                for k in range(NCH):
                    nc.tensor.matmul(
                        ps[:], T2T[k][:, ts(i, P)], PT[k][:],
                        start=False, stop=False,
                    )
                nc.tensor.matmul(
                    ps[:], ones_sb[:, :P], br_sb[:, :], start=False, stop=True
                )

                expb = exp_pool.tile([P, S], BF16, tag=f"expb{i}")
                z = zr_pool.tile([P, 1], F32, tag=f"z{i}")
                nc.scalar.activation(
                    expb[:], ps[:], mybir.ActivationFunctionType.Exp,
                    scale=float(NORM), accum_out=z[:],
                )
                r = zr_pool.tile([P, 1], F32, tag=f"r{i}")
                nc.vector.reciprocal(r[:], z[:])

                if USE_XBAR:
                    # launch the XBAR transpose DMA now; out matmuls two
                    # chunks later so the DMA latency is fully hidden
                    E = make_tp(i, expb)()
                    if len(pend) == 2:
                        pend.pop(0)()
                    pend.append(
                        lambda E=E, fn=make_out(i, b, r): fn(E)
                    )
                elif prev is not None:
                    prev[1](E_prev)

                asm = exp_pool.tile([P, S], F32, tag=f"asm{i}")
                nc.vector.tensor_scalar_mul(asm[:], expb[:], r[:])
                nc.sync.dma_start(att_d[b, ts(i, P), :], asm[:])

                if not USE_XBAR:
                    prev = (make_tp(i, expb), make_out(i, b, r))

            if USE_XBAR:
                # two outs still pending: emit the older now, defer the
                # last into the next batch's projection phase
                pend.pop(0)()
                tail_tp[0] = None
                tail_out[0] = pend.pop(0)
            else:
                # defer last chunk's tp/out into the next batch
                tp_fn, out_fn = prev
                E_box = [None]

                def mk_tail_tp(tp_fn=tp_fn, E_box=E_box):
                    def emit():
                        E_box[0] = tp_fn()

                    return emit

                def mk_tail_out(out_fn=out_fn, E_box=E_box):
                    def emit():
                        out_fn(E_box[0])

                    return emit

                tail_tp[0] = mk_tail_tp()
                tail_out[0] = mk_tail_out()

        # flush final batch's tail
        if tail_tp[0] is not None:
            tail_tp[0]()
        if tail_out[0] is not None:
            tail_out[0]()

    _split_multiwait(nc)
    return nc


_prog_cache = {}


def _get_program(with_gv: bool, repeat: int = 1):
    key = (with_gv, repeat)
    if key not in _prog_cache:
        _prog_cache[key] = _build_program(with_gv, repeat)
    return _prog_cache[key]


def _make_runner(nc, donate=True):
    """Persistent jitted SPMD runner (mirrors bass2jax.run_bass_via_pjrt but
    caches the jax.jit so repeat calls don't re-lower/re-compile)."""
    import jax
    from jax.experimental.shard_map import shard_map
    from jax.sharding import Mesh, PartitionSpec
    from concourse.bass2jax import (
        _bass_exec_p,
        install_neuronx_cc_hook,
        partition_id_tensor,
    )

    install_neuronx_cc_hook()
    partition_name = (
        nc.partition_id_tensor.name if nc.partition_id_tensor else None
    )
    in_names, out_names, out_avals, out_shapes = [], [], [], []
    for alloc in nc.m.functions[0].allocations:
        if not isinstance(alloc, mybir.MemoryLocationSet):
            continue
        name = alloc.memorylocations[0].name
        if alloc.kind == "ExternalInput":
            if name != partition_name:
                in_names.append(name)
        elif alloc.kind == "ExternalOutput":
            shape = tuple(alloc.tensor_shape)
            dtype = mybir.dt.np(alloc.dtype)
            out_names.append(name)
            out_avals.append(jax.core.ShapedArray(shape, dtype))
            out_shapes.append((shape, dtype))
    n_params = len(in_names)
    all_in_names = list(in_names) + list(out_names)
    if partition_name is not None:
        all_in_names.append(partition_name)
    donate = tuple(range(n_params, n_params + len(out_names))) if donate else ()

    def _body(*args):
        operands = list(args)
        if partition_name is not None:
            operands.append(partition_id_tensor())
        outs = _bass_exec_p.bind(
            *operands,
            out_avals=tuple(out_avals),
            in_names=tuple(all_in_names),
            out_names=tuple(out_names),
            lowering_input_output_aliases=(),
            sim_require_finite=True,
            sim_require_nnan=True,
            nc=nc,
        )
        return tuple(outs)

    devices = jax.devices()[:NCORES]
    mesh = Mesh(np.asarray(devices), ("core",))
    in_specs = (PartitionSpec("core"),) * (n_params + len(out_names))
    out_specs = (PartitionSpec("core"),) * len(out_names)
    sharded = jax.jit(
        shard_map(
            _body, mesh=mesh, in_specs=in_specs, out_specs=out_specs,
            check_rep=False,
        ),
        donate_argnums=donate if donate else (),
        keep_unused=True,
    )

    def prep(in_maps):
        per_core = [[np.asarray(m[name]) for name in in_names] for m in in_maps]
        concat_in = [
            np.concatenate([per_core[c][i] for c in range(NCORES)], axis=0)
            for i in range(n_params)
        ]
        concat_zeros = [
            np.zeros((NCORES * s[0], *s[1:]), d) for (s, d) in out_shapes
        ]
        return concat_in, concat_zeros

    def run(in_maps, as_numpy=True):
        concat_in, concat_zeros = prep(in_maps)
        out_arrs = sharded(*concat_in, *concat_zeros)
        if not as_numpy:
            jax.block_until_ready(out_arrs)
            return None
        return {n: np.asarray(out_arrs[i]) for i, n in enumerate(out_names)}

    run.sharded = sharded
    run.prep = prep
    run.mesh = mesh
    run.out_names = out_names
    return run


_runner_cache = {}


def _get_runner(with_gv: bool, repeat: int = 1):
    key = (with_gv, repeat)
    if key not in _runner_cache:
        _runner_cache[key] = _make_runner(_get_program(with_gv, repeat))
    return _runner_cache[key]


def _prepare(
    sent_emb, pos_emb, branch_emb,
    Wq, bq, Wk, bk, Wv, bv, Wpq, bpq, Wpk, bpk, bias,
):
    x = np.ascontiguousarray(np.asarray(sent_emb, dtype=np.float32)).astype(BF)
    p = np.ascontiguousarray(np.asarray(pos_emb, dtype=np.float32)).astype(BF)
    xt = np.ascontiguousarray(x.transpose(0, 2, 1))
    pt = np.ascontiguousarray(p.transpose(0, 2, 1))
    Wq = np.asarray(Wq, np.float32); Wk = np.asarray(Wk, np.float32)
    Wv = np.asarray(Wv, np.float32)
    Wpq = np.asarray(Wpq, np.float32); Wpk = np.asarray(Wpk, np.float32)
    bq = np.asarray(bq, np.float32); bk = np.asarray(bk, np.float32)
    bv = np.asarray(bv, np.float32)
    bpq = np.asarray(bpq, np.float32); bpk = np.asarray(bpk, np.float32)
    bias = np.asarray(bias, np.float32)

    W1 = (Wq.T @ Wk).astype(BF)
    W2 = (Wpq.T @ Wpk).astype(BF)
    wvt = np.ascontiguousarray(Wv.T).astype(BF)
    biasrow = (bias / NORM)[None, :].astype(BF)
    onesrow = np.ones((1, S), BF)

    g1 = Wk.T @ bq
    g2 = Wpk.T @ bpq
    with_gv = bool(np.any(g1) or np.any(g2) or np.any(bv))
    gv = np.concatenate([g1, g2, bv])[None, :].astype(BF)

    in_maps = []
    for c in range(NCORES):
        m = {
            "xt": xt[c * BPC : (c + 1) * BPC],
            "pt": pt[c * BPC : (c + 1) * BPC],
            "w1": W1, "w2": W2, "wvt": wvt,
            "biasrow": biasrow, "onesrow": onesrow,
        }
        if with_gv:
            m["gv"] = gv
        in_maps.append(m)
    return with_gv, in_maps


def kernel(**inputs):
    with_gv, in_maps = _prepare(**inputs)
    run = _get_runner(with_gv)
    outs = run(in_maps)
    return outs["att"], outs["out"]
